# revision 12
# baseline (speedup 1.0000x reference)
"""Trainium2 Bass kernel for nn_Decoder_6055903887927 (gnn_message_passing).

Math (per irrep i, d_i in (1,3,5)):
  h = silu(silu(inv @ w1 + b1) @ w2 + b2)
  r2f = (h @ w3 + b3) * 1/sqrt(RBF)            # (A, RBF, F)
  sparse[t,f] += sum_{n,d,r} sph_i[n,t,d] * feat_i[n,f,d]
                             * rbf[n,t,r] * r2f[n,r,f]
  out[idx[t]] += sparse[t]                     # scatter-add into (N, F)

Strategy: CP-expansion of the einsum into one big matmul with contraction
axis K = (d, r, n) of size 9*16*128 = 18432:
  sparse^T[f, t] = sum_K W[K, f] * P[K, t]
  W[(d,r,n), f] = feat_d[n, f] * r2f[n, r, f]     (elementwise build, DVE)
  P[(d,r,n), t] = sph_d[n, t] * rbf_r[n, t]       (elementwise build, DVE)
Both builds have n on partitions -> matmul contracts partitions directly.
The builds and the matmul run in fp16 (inputs cast host-side), accumulation
stays fp32 in PSUM.

Sharding: split the TARGET axis t (T=2048) across 8 cores (TL=256 each).
The einsum reduces over (n, d, r), not t, so each core's rows are complete -
no all-reduce. The per-core TL is further split into two halves of 128 so
the first AllGather (64KB/rank) and its ~12us ncfw wake latency overlap the
second half's matmuls. After each AG every core holds that half's full
(T/2, F) result as columns [f, t]; duplicate targets are merged with
statically-emitted column adds (the program is compiled per idx), tiles are
transposed back to row-major on the PE, and each core scatters the rows
whose target falls in its NR=8192-row output shard with indirect DMAs.
Non-owned / duplicate rows scatter to a trash row (row NR) that the host
drops - no OOB semantics needed.
"""

import sys

sys.path.insert(0, "/opt/trn_rl_repo")

import numpy as np

import concourse.bass as bass
import concourse.mybir as mybir
from concourse import bacc, tile
from concourse.masks import make_identity

A, T, NGRID, RBF, F = 128, 2048, 65536, 16, 128
DS = (1, 3, 5)
NDP = sum(DS)  # 9 d-planes
NCORES = 8
TL = T // NCORES  # 256 targets per core
HL = TL // 2  # 128 targets per half
NR = NGRID // NCORES  # 8192 output rows per core
NTILE = NCORES  # column tiles of 128 per half

F32 = mybir.dt.float32
BF16 = mybir.dt.bfloat16
F16 = mybir.dt.float16
I32 = mybir.dt.int32
MD_MAP = {"f32": F32, "bf16": BF16, "f16": F16}

# dtype of the P/W builds, the big matmul, and MLP layer 3.
MAIN_DTYPE = "f16"
# scatter mode: "multi" = one indirect DMA per half (offsets [128, 8]),
# "per_tile" = one indirect DMA per 128 rows.
SCATTER = "per_tile"

_CACHE: dict = {}


def _t_decomp(t):
    """global t -> (half, tile j, partition p); column in cols_h = j*128+p."""
    c, rem = divmod(t, TL)
    h, p = divmod(rem, HL)
    return h, c, p


def _build_program(dup_pairs, b3_nonzero, main_dtype, repeats=1):
    md = MD_MAP[main_dtype]
    nc = bacc.Bacc(
        "TRN2", target_bir_lowering=False, debug=False, num_devices=NCORES
    )

    invT_h = nc.dram_tensor("invT", [F, A], F32, kind="ExternalInput")
    wmat_h = nc.dram_tensor("wmat", [6, F, F], F32, kind="ExternalInput")
    bvec_h = nc.dram_tensor("bvec", [6, F, 1], F32, kind="ExternalInput")
    w3_h = nc.dram_tensor("w3", [3, F, RBF * F], md, kind="ExternalInput")
    if b3_nonzero:
        b3_h = nc.dram_tensor("b3", [3, 1, RBF * F], md, kind="ExternalInput")
    featp_h = nc.dram_tensor("featp", [NDP, A, F], md, kind="ExternalInput")
    planes_h = nc.dram_tensor(
        "planes", [NDP + RBF, A, TL], md, kind="ExternalInput"
    )
    offs_h = nc.dram_tensor("offs", [128, T // 128], I32, kind="ExternalInput")
    out_h = nc.dram_tensor("out", [NR + 1, F], F32, kind="ExternalOutput")

    with tile.TileContext(nc) as tc:
        with (
            tc.tile_pool(name="const", bufs=1) as const,
            tc.tile_pool(name="mlp", bufs=2) as mlp,
            tc.tile_pool(name="work", bufs=2) as work,
            tc.tile_pool(name="psacc", bufs=1, space="PSUM") as psacc,
            tc.tile_pool(name="pssm", bufs=1, space="PSUM") as pssm,
            tc.tile_pool(name="psbig", bufs=1, space="PSUM") as psbig,
            tc.tile_pool(name="pstr", bufs=2, space="PSUM") as pstr,
            tc.tile_pool(name="dram", bufs=1, space="DRAM") as dram,
        ):
            # ---- input loads (few big DMAs; planes first) ----
            planes_t = const.tile([A, (NDP + RBF) * TL], md, tag="planes")
            nc.sync.dma_start(
                planes_t[:].rearrange("p (n t) -> p n t", n=NDP + RBF),
                planes_h[:].transpose([1, 0, 2]),
            )
            featp_t = const.tile([A, NDP * F], md, tag="featp")
            nc.sync.dma_start(
                featp_t[:].rearrange("p (n f) -> p n f", n=NDP),
                featp_h[:].transpose([1, 0, 2]),
            )
            w3_t = const.tile([F, 3 * RBF * F], md, tag="w3")
            nc.sync.dma_start(
                w3_t[:].rearrange("p (i j) -> p i j", i=3),
                w3_h[:].transpose([1, 0, 2]),
            )
            invT_t = const.tile([F, A], F32, tag="invT")
            nc.sync.dma_start(invT_t[:], invT_h[:])
            wmat_t = const.tile([F, 6 * F], F32, tag="wmat")
            nc.sync.dma_start(
                wmat_t[:].rearrange("p (i j) -> p i j", i=6),
                wmat_h[:].transpose([1, 0, 2]),
            )
            bvec_t = const.tile([F, 6], F32, tag="bvec")
            nc.sync.dma_start(
                bvec_t[:].rearrange("p (i j) -> p i j", i=6),
                bvec_h[:].transpose([1, 0, 2]),
            )
            offs_t = const.tile([128, T // 128], I32, tag="offs")
            nc.sync.dma_start(offs_t[:], offs_h[:])
            if b3_nonzero:
                b3_t = const.tile([1, 3 * RBF * F], md, tag="b3")
                nc.sync.dma_start(
                    b3_t[:].rearrange("p (i j) -> p i j", i=3),
                    b3_h[:].transpose([1, 0, 2]),
                )
                ones_t = const.tile([1, A], md, tag="ones")
                nc.gpsimd.memset(ones_t[:], 1.0)
            ident = const.tile([128, 128], F32, tag="ident")
            make_identity(nc, ident[:])

            def sph(dg):
                return planes_t[:, dg * TL:(dg + 1) * TL]

            def rbf_all():
                return planes_t[:, NDP * TL:].rearrange(
                    "p (r t) -> p r t", r=RBF
                )

            for _rep in range(repeats):
                # ---- MLP (per irrep): r2f = silu(silu(inv@w1+b1)@w2+b2)@w3
                r2f_t = []
                for i in range(3):
                    h1p = pssm.tile([F, A], F32, tag="hsm", bufs=2, name="h1p")
                    nc.tensor.matmul(
                        h1p[:], wmat_t[:, i * F:(i + 1) * F], invT_t[:],
                        start=True, stop=True,
                    )
                    h1 = mlp.tile([F, A], F32, tag="h1")
                    nc.scalar.activation(
                        h1[:], h1p[:], mybir.ActivationFunctionType.Silu,
                        bias=bvec_t[:, i:i + 1],
                    )
                    h2p = pssm.tile([F, A], F32, tag="hsm", bufs=2, name="h2p")
                    nc.tensor.matmul(
                        h2p[:], wmat_t[:, (3 + i) * F:(4 + i) * F], h1[:],
                        start=True, stop=True,
                    )
                    h2 = mlp.tile([F, A], md, tag="h2")
                    nc.scalar.activation(
                        h2[:], h2p[:], mybir.ActivationFunctionType.Silu,
                        bias=bvec_t[:, 3 + i:4 + i],
                    )
                    r2f = mlp.tile([A, RBF * F], md, tag=f"r2f_{i}")
                    for jh in range(2):
                        r2fp = psbig.tile([A, 1024], F32, tag="r2fp")
                        for j in range(2):
                            sl = slice(
                                i * RBF * F + jh * 1024 + j * 512,
                                i * RBF * F + jh * 1024 + (j + 1) * 512,
                            )
                            psl = slice(j * 512, (j + 1) * 512)
                            nc.tensor.matmul(
                                r2fp[:, psl], h2[:], w3_t[:, sl],
                                start=True, stop=not b3_nonzero,
                            )
                            if b3_nonzero:
                                nc.tensor.matmul(
                                    r2fp[:, psl], ones_t[:],
                                    b3_t[:, i * RBF * F + jh * 1024 + j * 512:
                                         i * RBF * F + jh * 1024 + (j + 1) * 512],
                                    start=False, stop=True,
                                )
                        nc.scalar.activation(
                            r2f[:, jh * 1024:(jh + 1) * 1024], r2fp[:],
                            mybir.ActivationFunctionType.Copy,
                        )
                    r2f_t.append(r2f)

                # ---- DVE: build all P / W blocks (resident) ----
                pb_t, wb_t = [], []
                dg = 0
                for i in range(3):
                    for d in range(DS[i]):
                        pb = const.tile(
                            [A, RBF, TL], md, tag=f"pb{dg}", name=f"pb{dg}"
                        )
                        nc.vector.tensor_mul(
                            pb[:], rbf_all(),
                            sph(dg).unsqueeze(1).broadcast_to([A, RBF, TL]),
                        )
                        wb = const.tile(
                            [A, RBF, F], md, tag=f"wb{dg}", name=f"wb{dg}"
                        )
                        nc.vector.tensor_mul(
                            wb[:],
                            r2f_t[i][:].rearrange("p (r f) -> p r f", r=RBF),
                            featp_t[:, dg * F:(dg + 1) * F]
                            .unsqueeze(1).broadcast_to([A, RBF, F]),
                        )
                        pb_t.append(pb)
                        wb_t.append(wb)
                        dg += 1

                # ---- PE: contraction per half; AG0 overlaps half 1 ----
                cols, agouts = [], []
                for h in range(2):
                    acc = psacc.tile(
                        [F, HL], F32, tag=f"acc{h}", name=f"acc{h}"
                    )
                    it = 0
                    for dg in range(NDP):
                        for r in range(RBF):
                            nc.tensor.matmul(
                                acc[:],
                                wb_t[dg][:, r, :],
                                pb_t[dg][:, r, h * HL:(h + 1) * HL],
                                start=(it == 0), stop=(it == NDP * RBF - 1),
                            )
                            it += 1
                    accs = work.tile([F, HL], F32, tag="accs")
                    nc.scalar.activation(
                        accs[:], acc[:], mybir.ActivationFunctionType.Copy
                    )
                    agin = dram.tile([F, HL], F32, name=f"agin{h}")
                    nc.gpsimd.dma_start(agin[:], accs[:])
                    agout = dram.tile([NCORES * F, HL], F32, name=f"agout{h}")
                    nc.gpsimd.collective_compute(
                        "AllGather",
                        mybir.AluOpType.bypass,
                        replica_groups=[list(range(NCORES))],
                        ins=[agin[:].opt()],
                        outs=[agout[:].opt()],
                    )
                    agouts.append(agout)
                    cols.append(
                        const.tile([F, NCORES * HL], F32, tag=f"cols{h}",
                                   name=f"cols{h}")
                    )

                # load each half's gathered columns tile-by-tile
                for h in range(2):
                    for j in range(NTILE):
                        nc.sync.dma_start(
                            cols[h][:, j * HL:(j + 1) * HL],
                            agouts[h][j * F:(j + 1) * F, :],
                        )

                # ---- static dup merges (column adds) ----
                for lt, dt_ in dup_pairs:
                    lh, lj, lp = _t_decomp(lt)
                    dh, dj, dp = _t_decomp(dt_)
                    lcol = lj * HL + lp
                    dcol = dj * HL + dp
                    nc.vector.tensor_add(
                        cols[lh][:, lcol:lcol + 1],
                        cols[lh][:, lcol:lcol + 1],
                        cols[dh][:, dcol:dcol + 1],
                    )

                # ---- transpose to rows and scatter owned rows ----
                for h in range(2):
                    rows = work.tile(
                        [128, NTILE, F], F32, tag=f"rows{h}", name=f"rows{h}"
                    )
                    for j in range(NTILE):
                        trp = pstr.tile([128, 128], F32, tag="trp", bufs=2)
                        nc.tensor.transpose(
                            trp[:], cols[h][:, j * HL:(j + 1) * HL], ident[:]
                        )
                        nc.scalar.activation(
                            rows[:, j, :], trp[:],
                            mybir.ActivationFunctionType.Copy,
                        )
                        if SCATTER == "per_tile":
                            nc.gpsimd.indirect_dma_start(
                                out=out_h[:],
                                out_offset=bass.IndirectOffsetOnAxis(
                                    ap=offs_t[
                                        :, h * NTILE + j:h * NTILE + j + 1
                                    ],
                                    axis=0,
                                ),
                                in_=rows[:, j, :],
                                in_offset=None,
                            )
                    if SCATTER == "multi":
                        nc.gpsimd.indirect_dma_start(
                            out=out_h[:],
                            out_offset=bass.IndirectOffsetOnAxis(
                                ap=offs_t[:, h * NTILE:(h + 1) * NTILE],
                                axis=0,
                            ),
                            in_=rows[:],
                            in_offset=None,
                        )

    nc.compile()
    return nc


def _prep(inputs, main_dtype):
    """Host-side input prep -> (per-core in_maps, dup_pairs, b3_nonzero)."""
    if main_dtype == "f32":
        md = np.float32
    elif main_dtype == "f16":
        md = np.float16
    else:
        import ml_dtypes

        md = np.dtype(ml_dtypes.bfloat16)

    f0 = np.asarray(inputs["feat0"], np.float32)
    inv_rbf = np.float32(1.0 / np.sqrt(RBF))

    invT = np.ascontiguousarray(f0[:, :, 0].T)
    w1 = np.asarray(inputs["mlp_w1"], np.float32)
    w2 = np.asarray(inputs["mlp_w2"], np.float32)
    wmat = np.ascontiguousarray(np.concatenate([w1, w2], axis=0))
    b1 = np.asarray(inputs["mlp_b1"], np.float32)
    b2 = np.asarray(inputs["mlp_b2"], np.float32)
    bvec = np.concatenate([b1, b2], axis=0).reshape(6, F, 1).copy()
    w3 = (np.asarray(inputs["mlp_w3"], np.float32) * inv_rbf).astype(md)
    b3f = np.asarray(inputs["mlp_b3"], np.float32) * inv_rbf
    b3_nonzero = bool(np.any(b3f))
    b3 = b3f.reshape(3, 1, RBF * F).astype(md)

    featp = np.concatenate(
        [
            np.asarray(inputs[f"feat{i}"], np.float32).transpose(2, 0, 1)
            for i in range(3)
        ],
        axis=0,
    ).astype(md)  # (9, A, F)
    sphp = np.concatenate(
        [
            np.asarray(inputs[f"sph{i}"], np.float32).transpose(2, 0, 1)
            for i in range(3)
        ],
        axis=0,
    )  # (9, A, T)
    rbfp = np.asarray(inputs["radial_basis_vals"], np.float32).transpose(
        2, 0, 1
    )  # (RBF, A, T)
    planes = np.concatenate([sphp, rbfp], axis=0).astype(md)  # (25, A, T)

    idx = np.asarray(inputs["truncated_idx"]).astype(np.int64)
    first: dict = {}
    dup_pairs = []
    for t, v in enumerate(idx.tolist()):
        if v in first:
            dup_pairs.append((first[v], t))
        else:
            first[v] = t

    in_maps = []
    for c in range(NCORES):
        # offs2d[p, h*8+j] = offset for global t = j*TL + h*HL + p
        off_t = np.full(T, NR, np.int64)  # default: trash row
        lo, hi = c * NR, (c + 1) * NR
        for v, lt in first.items():
            if lo <= v < hi:
                off_t[lt] = v - lo
        offs2d = np.empty((128, T // 128), np.int32)
        for h in range(2):
            for j in range(NTILE):
                base = j * TL + h * HL
                offs2d[:, h * NTILE + j] = off_t[base:base + HL]
        ts = slice(c * TL, (c + 1) * TL)
        m = {
            "invT": invT,
            "wmat": wmat, "bvec": bvec, "w3": w3,
            "featp": featp,
            "planes": np.ascontiguousarray(planes[:, :, ts]),
            "offs": np.ascontiguousarray(offs2d),
        }
        if b3_nonzero:
            m["b3"] = b3
        in_maps.append(m)
    return in_maps, tuple(dup_pairs), b3_nonzero


def _get_runner(dup_pairs, b3_nonzero, main_dtype, repeats=1):
    key = (dup_pairs, b3_nonzero, main_dtype, repeats, SCATTER)
    if key not in _CACHE:
        nc = _build_program(dup_pairs, b3_nonzero, main_dtype, repeats)
        _CACHE[key] = nc
    return _CACHE[key]


def run_on_hw(in_maps, nc):
    from concourse import bass_utils

    res = bass_utils.run_bass_kernel_spmd(
        nc, in_maps, core_ids=list(range(NCORES))
    )
    return res.results


def kernel(**inputs) -> np.ndarray:
    in_maps, dup_pairs, b3_nonzero = _prep(inputs, MAIN_DTYPE)
    nc = _get_runner(dup_pairs, b3_nonzero, MAIN_DTYPE)
    results = run_on_hw(in_maps, nc)
    return np.concatenate(
        [results[c]["out"][:NR] for c in range(NCORES)], axis=0
    )


# revision 14
# speedup vs baseline: 1.5584x; 1.5584x over previous
"""Trainium2 Bass kernel for nn_Decoder_6055903887927 (gnn_message_passing).

Math (per irrep i, d_i in (1,3,5)):
  h = silu(silu(inv @ w1 + b1) @ w2 + b2)
  r2f = (h @ w3 + b3) * 1/sqrt(RBF)            # (A, RBF, F)
  sparse[t,f] += sum_{n,d,r} sph_i[n,t,d] * feat_i[n,f,d]
                             * rbf[n,t,r] * r2f[n,r,f]
  out[idx[t]] += sparse[t]                     # scatter-add into (N, F)

Strategy: CP-expansion of the einsum into one big matmul with contraction
axis K = (d, r, n) of size 9*16*128 = 18432:
  sparse^T[f, t] = sum_K W[K, f] * P[K, t]
  W[(d,r,n), f] = feat_d[n, f] * r2f[n, r, f]     (elementwise build, DVE)
  P[(d,r,n), t] = sph_d[n, t] * rbf_r[n, t]       (elementwise build, DVE)
Both builds have n on partitions -> matmul contracts partitions directly.
The builds and the matmul run in fp16 (inputs cast host-side), accumulation
stays fp32 in PSUM.

Sharding: split the TARGET axis t (T=2048) across 8 cores (TL=256 each).
The einsum reduces over (n, d, r), not t, so each core's rows are complete -
no all-reduce is needed, only one small AllGather (129KB/rank, row-major
with an explicit zero row appended per rank). The program is compiled per
truncated_idx: each core compacts the ~NR-owned target rows out of the
gathered (T, F) table with a few indirect gathers (duplicate targets pull
their partners from per-level tables, empty slots read the zero row, and
one vector add per level merges them), then scatters the compacted rows
into its NR=8192-row shard of the output with 2-3 indirect DMAs. Pad slots
scatter to a trash row (row NR) that the host drops; scatters alternate
between two output buffers so they pipeline, and the host sums the two.
"""

import sys

sys.path.insert(0, "/opt/trn_rl_repo")

import numpy as np

import concourse.bass as bass
import concourse.mybir as mybir
from concourse import bacc, tile
from concourse.masks import make_identity

A, T, NGRID, RBF, F = 128, 2048, 65536, 16, 128
DS = (1, 3, 5)
NDP = sum(DS)  # 9 d-planes
NCORES = 8
TL = T // NCORES  # 256 targets per core
HL = TL // 2  # 128 targets per half
NR = NGRID // NCORES  # 8192 output rows per core
NTILE = NCORES  # column tiles of 128 per half

F32 = mybir.dt.float32
BF16 = mybir.dt.bfloat16
F16 = mybir.dt.float16
I32 = mybir.dt.int32
MD_MAP = {"f32": F32, "bf16": BF16, "f16": F16}

# dtype of the P/W builds, the big matmul, and MLP layer 3.
MAIN_DTYPE = "f16"
_CACHE: dict = {}


def _build_program(b3_nonzero, main_dtype, n_main, n_lvls, repeats=1):
    md = MD_MAP[main_dtype]
    nc = bacc.Bacc(
        "TRN2", target_bir_lowering=False, debug=False, num_devices=NCORES
    )

    invT_h = nc.dram_tensor("invT", [F, A], F32, kind="ExternalInput")
    wmat_h = nc.dram_tensor("wmat", [6, F, F], F32, kind="ExternalInput")
    bvec_h = nc.dram_tensor("bvec", [6, F, 1], F32, kind="ExternalInput")
    w3_h = nc.dram_tensor("w3", [3, F, RBF * F], md, kind="ExternalInput")
    if b3_nonzero:
        b3_h = nc.dram_tensor("b3", [3, 1, RBF * F], md, kind="ExternalInput")
    featp_h = nc.dram_tensor("featp", [NDP, A, F], md, kind="ExternalInput")
    planes_h = nc.dram_tensor(
        "planes", [NDP + RBF, A, TL], md, kind="ExternalInput"
    )
    gidx_h = nc.dram_tensor(
        "gidx", [128, n_main * (1 + n_lvls)], I32, kind="ExternalInput"
    )
    soffs_h = nc.dram_tensor("soffs", [128, n_main], I32, kind="ExternalInput")
    out_h = nc.dram_tensor("out", [NR + 1, F], F32, kind="ExternalOutput")
    outb_h = nc.dram_tensor("outb", [NR + 1, F], F32, kind="ExternalOutput")

    with tile.TileContext(nc) as tc:
        with (
            tc.tile_pool(name="const", bufs=1) as const,
            tc.tile_pool(name="mlp", bufs=2) as mlp,
            tc.tile_pool(name="work", bufs=2) as work,
            tc.tile_pool(name="psacc", bufs=1, space="PSUM") as psacc,
            tc.tile_pool(name="pssm", bufs=1, space="PSUM") as pssm,
            tc.tile_pool(name="psbig", bufs=1, space="PSUM") as psbig,
            tc.tile_pool(name="pstr", bufs=2, space="PSUM") as pstr,
            tc.tile_pool(name="dram", bufs=1, space="DRAM") as dram,
        ):
            # ---- input loads (few big DMAs; planes first) ----
            planes_t = const.tile([A, (NDP + RBF) * TL], md, tag="planes")
            nc.sync.dma_start(
                planes_t[:].rearrange("p (n t) -> p n t", n=NDP + RBF),
                planes_h[:].transpose([1, 0, 2]),
            )
            featp_t = const.tile([A, NDP * F], md, tag="featp")
            nc.sync.dma_start(
                featp_t[:].rearrange("p (n f) -> p n f", n=NDP),
                featp_h[:].transpose([1, 0, 2]),
            )
            w3_t = const.tile([F, 3 * RBF * F], md, tag="w3")
            nc.sync.dma_start(
                w3_t[:].rearrange("p (i j) -> p i j", i=3),
                w3_h[:].transpose([1, 0, 2]),
            )
            invT_t = const.tile([F, A], F32, tag="invT")
            nc.sync.dma_start(invT_t[:], invT_h[:])
            wmat_t = const.tile([F, 6 * F], F32, tag="wmat")
            nc.sync.dma_start(
                wmat_t[:].rearrange("p (i j) -> p i j", i=6),
                wmat_h[:].transpose([1, 0, 2]),
            )
            bvec_t = const.tile([F, 6], F32, tag="bvec")
            nc.sync.dma_start(
                bvec_t[:].rearrange("p (i j) -> p i j", i=6),
                bvec_h[:].transpose([1, 0, 2]),
            )
            gidx_t = const.tile([128, n_main * (1 + n_lvls)], I32, tag="gidx")
            nc.sync.dma_start(gidx_t[:], gidx_h[:])
            soffs_t = const.tile([128, n_main], I32, tag="soffs")
            nc.sync.dma_start(soffs_t[:], soffs_h[:])
            zrow_t = const.tile([1, F], F32, tag="zrow")
            nc.vector.memset(zrow_t[:], 0.0)
            if b3_nonzero:
                b3_t = const.tile([1, 3 * RBF * F], md, tag="b3")
                nc.sync.dma_start(
                    b3_t[:].rearrange("p (i j) -> p i j", i=3),
                    b3_h[:].transpose([1, 0, 2]),
                )
                ones_t = const.tile([1, A], md, tag="ones")
                nc.gpsimd.memset(ones_t[:], 1.0)
            ident = const.tile([128, 128], F32, tag="ident")
            make_identity(nc, ident[:])

            def sph(dg):
                return planes_t[:, dg * TL:(dg + 1) * TL]

            def rbf_all():
                return planes_t[:, NDP * TL:].rearrange(
                    "p (r t) -> p r t", r=RBF
                )

            for _rep in range(repeats):
                # ---- MLP (per irrep): r2f = silu(silu(inv@w1+b1)@w2+b2)@w3
                r2f_t = []
                for i in range(3):
                    h1p = pssm.tile([F, A], F32, tag="hsm", bufs=2, name="h1p")
                    nc.tensor.matmul(
                        h1p[:], wmat_t[:, i * F:(i + 1) * F], invT_t[:],
                        start=True, stop=True,
                    )
                    h1 = mlp.tile([F, A], F32, tag="h1")
                    nc.scalar.activation(
                        h1[:], h1p[:], mybir.ActivationFunctionType.Silu,
                        bias=bvec_t[:, i:i + 1],
                    )
                    h2p = pssm.tile([F, A], F32, tag="hsm", bufs=2, name="h2p")
                    nc.tensor.matmul(
                        h2p[:], wmat_t[:, (3 + i) * F:(4 + i) * F], h1[:],
                        start=True, stop=True,
                    )
                    h2 = mlp.tile([F, A], md, tag="h2")
                    nc.scalar.activation(
                        h2[:], h2p[:], mybir.ActivationFunctionType.Silu,
                        bias=bvec_t[:, 3 + i:4 + i],
                    )
                    r2f = mlp.tile([A, RBF * F], md, tag=f"r2f_{i}")
                    for jh in range(2):
                        r2fp = psbig.tile([A, 1024], F32, tag="r2fp")
                        for j in range(2):
                            sl = slice(
                                i * RBF * F + jh * 1024 + j * 512,
                                i * RBF * F + jh * 1024 + (j + 1) * 512,
                            )
                            psl = slice(j * 512, (j + 1) * 512)
                            nc.tensor.matmul(
                                r2fp[:, psl], h2[:], w3_t[:, sl],
                                start=True, stop=not b3_nonzero,
                            )
                            if b3_nonzero:
                                nc.tensor.matmul(
                                    r2fp[:, psl], ones_t[:],
                                    b3_t[:, i * RBF * F + jh * 1024 + j * 512:
                                         i * RBF * F + jh * 1024 + (j + 1) * 512],
                                    start=False, stop=True,
                                )
                        nc.scalar.activation(
                            r2f[:, jh * 1024:(jh + 1) * 1024], r2fp[:],
                            mybir.ActivationFunctionType.Copy,
                        )
                    r2f_t.append(r2f)

                # ---- DVE: build all P / W blocks (resident) ----
                pb_t, wb_t = [], []
                dg = 0
                for i in range(3):
                    for d in range(DS[i]):
                        pb = const.tile(
                            [A, RBF, TL], md, tag=f"pb{dg}", name=f"pb{dg}"
                        )
                        nc.vector.tensor_mul(
                            pb[:], rbf_all(),
                            sph(dg).unsqueeze(1).broadcast_to([A, RBF, TL]),
                        )
                        wb = const.tile(
                            [A, RBF, F], md, tag=f"wb{dg}", name=f"wb{dg}"
                        )
                        nc.vector.tensor_mul(
                            wb[:],
                            r2f_t[i][:].rearrange("p (r f) -> p r f", r=RBF),
                            featp_t[:, dg * F:(dg + 1) * F]
                            .unsqueeze(1).broadcast_to([A, RBF, F]),
                        )
                        pb_t.append(pb)
                        wb_t.append(wb)
                        dg += 1

                # ---- PE: single-pass contraction (144 matmuls, N=256) ----
                acc = psacc.tile([F, TL], F32, tag="acc")
                it = 0
                for dg in range(NDP):
                    for r in range(RBF):
                        nc.tensor.matmul(
                            acc[:],
                            wb_t[dg][:, r, :],
                            pb_t[dg][:, r, :],
                            start=(it == 0), stop=(it == NDP * RBF - 1),
                        )
                        it += 1
                accs = work.tile([F, TL], F32, tag="accs")
                nc.scalar.activation(
                    accs[:], acc[:], mybir.ActivationFunctionType.Copy
                )

                # ---- local transpose to row-major + zero row, then AG ----
                agin = dram.tile([TL + 1, F], F32, name="agin")
                for hh in range(2):
                    trp = pstr.tile([128, 128], F32, tag="trp", bufs=2)
                    nc.tensor.transpose(
                        trp[:], accs[:, hh * 128:(hh + 1) * 128], ident[:]
                    )
                    rl = work.tile([128, F], F32, tag="rl")
                    nc.scalar.activation(
                        rl[:], trp[:], mybir.ActivationFunctionType.Copy
                    )
                    nc.gpsimd.dma_start(
                        agin[hh * 128:(hh + 1) * 128, :], rl[:]
                    )
                nc.gpsimd.dma_start(agin[TL:TL + 1, :], zrow_t[:])
                agout = dram.tile([NCORES * (TL + 1), F], F32, name="agout")
                nc.gpsimd.collective_compute(
                    "AllGather",
                    mybir.AluOpType.bypass,
                    replica_groups=[list(range(NCORES))],
                    ins=[agin[:].opt()],
                    outs=[agout[:].opt()],
                )

                # ---- compact owned rows via indirect gathers, merge dups,
                # ---- scatter to the output shard (pads go to trash row NR)
                for g in range(n_main):
                    gm = work.tile([128, F], F32, tag=f"gm{g}", name=f"gm{g}")
                    nc.gpsimd.indirect_dma_start(
                        out=gm[:],
                        out_offset=None,
                        in_=agout[:],
                        in_offset=bass.IndirectOffsetOnAxis(
                            ap=gidx_t[:, g:g + 1], axis=0
                        ),
                    )
                    for l in range(n_lvls):
                        gd = work.tile(
                            [128, F], F32, tag=f"gd{g}_{l}", name=f"gd{g}_{l}"
                        )
                        nc.gpsimd.indirect_dma_start(
                            out=gd[:],
                            out_offset=None,
                            in_=agout[:],
                            in_offset=bass.IndirectOffsetOnAxis(
                                ap=gidx_t[
                                    :, n_main + l * n_main + g:
                                    n_main + l * n_main + g + 1
                                ],
                                axis=0,
                            ),
                        )
                        nc.vector.tensor_add(gm[:], gm[:], gd[:])
                    nc.gpsimd.indirect_dma_start(
                        out=(out_h if g % 2 == 0 else outb_h)[:],
                        out_offset=bass.IndirectOffsetOnAxis(
                            ap=soffs_t[:, g:g + 1], axis=0
                        ),
                        in_=gm[:],
                        in_offset=None,
                    )

    nc.compile()
    return nc


def _prep(inputs, main_dtype):
    """Host-side input prep -> (per-core in_maps, dup_pairs, b3_nonzero)."""
    if main_dtype == "f32":
        md = np.float32
    elif main_dtype == "f16":
        md = np.float16
    else:
        import ml_dtypes

        md = np.dtype(ml_dtypes.bfloat16)

    f0 = np.asarray(inputs["feat0"], np.float32)
    inv_rbf = np.float32(1.0 / np.sqrt(RBF))

    invT = np.ascontiguousarray(f0[:, :, 0].T)
    w1 = np.asarray(inputs["mlp_w1"], np.float32)
    w2 = np.asarray(inputs["mlp_w2"], np.float32)
    wmat = np.ascontiguousarray(np.concatenate([w1, w2], axis=0))
    b1 = np.asarray(inputs["mlp_b1"], np.float32)
    b2 = np.asarray(inputs["mlp_b2"], np.float32)
    bvec = np.concatenate([b1, b2], axis=0).reshape(6, F, 1).copy()
    w3 = (np.asarray(inputs["mlp_w3"], np.float32) * inv_rbf).astype(md)
    b3f = np.asarray(inputs["mlp_b3"], np.float32) * inv_rbf
    b3_nonzero = bool(np.any(b3f))
    b3 = b3f.reshape(3, 1, RBF * F).astype(md)

    featp = np.concatenate(
        [
            np.asarray(inputs[f"feat{i}"], np.float32).transpose(2, 0, 1)
            for i in range(3)
        ],
        axis=0,
    ).astype(md)  # (9, A, F)
    sphp = np.concatenate(
        [
            np.asarray(inputs[f"sph{i}"], np.float32).transpose(2, 0, 1)
            for i in range(3)
        ],
        axis=0,
    )  # (9, A, T)
    rbfp = np.asarray(inputs["radial_basis_vals"], np.float32).transpose(
        2, 0, 1
    )  # (RBF, A, T)
    planes = np.concatenate([sphp, rbfp], axis=0).astype(md)  # (25, A, T)

    idx = np.asarray(inputs["truncated_idx"]).astype(np.int64)
    first: dict = {}
    dups_of: dict = {}
    for t, v in enumerate(idx.tolist()):
        if v in first:
            dups_of.setdefault(v, []).append(t)
        else:
            first[v] = t
    n_lvls = max((len(x) for x in dups_of.values()), default=0)

    def agrow(t):
        return (t // TL) * (TL + 1) + (t % TL)

    ZROW = TL  # block-0 zero row in agout
    per_core_owned = []
    for c in range(NCORES):
        lo, hi = c * NR, (c + 1) * NR
        per_core_owned.append(sorted(v for v in first if lo <= v < hi))
    n_main = max(1, max((len(o) + 127) // 128 for o in per_core_owned))

    in_maps = []
    for c in range(NCORES):
        owned = per_core_owned[c]
        gidx = np.zeros((128, n_main * (1 + n_lvls)), np.int32)
        gidx[:, n_main:] = ZROW  # dup tables default to the zero row
        soffs = np.full((128, n_main), NR, np.int32)  # pads -> trash row
        for s, v in enumerate(owned):
            g, p = divmod(s, 128)
            gidx[p, g] = agrow(first[v])
            soffs[p, g] = v - c * NR
            for l, td in enumerate(dups_of.get(v, ())):
                gidx[p, n_main + l * n_main + g] = agrow(td)
        ts = slice(c * TL, (c + 1) * TL)
        m = {
            "invT": invT,
            "wmat": wmat, "bvec": bvec, "w3": w3,
            "featp": featp,
            "planes": np.ascontiguousarray(planes[:, :, ts]),
            "gidx": gidx,
            "soffs": soffs,
        }
        if b3_nonzero:
            m["b3"] = b3
        in_maps.append(m)
    return in_maps, (n_main, n_lvls), b3_nonzero



def _get_runner(meta, b3_nonzero, main_dtype, repeats=1):
    n_main, n_lvls = meta
    key = (meta, b3_nonzero, main_dtype, repeats)
    if key not in _CACHE:
        nc = _build_program(b3_nonzero, main_dtype, n_main, n_lvls, repeats)
        _CACHE[key] = nc
    return _CACHE[key]


def run_on_hw(in_maps, nc):
    from concourse import bass_utils

    res = bass_utils.run_bass_kernel_spmd(
        nc, in_maps, core_ids=list(range(NCORES))
    )
    return res.results


def kernel(**inputs) -> np.ndarray:
    in_maps, meta, b3_nonzero = _prep(inputs, MAIN_DTYPE)
    nc = _get_runner(meta, b3_nonzero, MAIN_DTYPE)
    results = run_on_hw(in_maps, nc)
    return np.concatenate(
        [
            results[c]["out"][:NR] + results[c]["outb"][:NR]
            for c in range(NCORES)
        ],
        axis=0,
    )


# revision 16
# speedup vs baseline: 1.6388x; 1.0516x over previous
"""Trainium2 Bass kernel for nn_Decoder_6055903887927 (gnn_message_passing).

Math (per irrep i, d_i in (1,3,5)):
  h = silu(silu(inv @ w1 + b1) @ w2 + b2)
  r2f = (h @ w3 + b3) * 1/sqrt(RBF)            # (A, RBF, F)
  sparse[t,f] += sum_{n,d,r} sph_i[n,t,d] * feat_i[n,f,d]
                             * rbf[n,t,r] * r2f[n,r,f]
  out[idx[t]] += sparse[t]                     # scatter-add into (N, F)

Strategy: CP-expansion of the einsum into one big matmul with contraction
axis K = (d, r, n) of size 9*16*128 = 18432:
  sparse^T[f, t] = sum_K W[K, f] * P[K, t]
  W[(d,r,n), f] = feat_d[n, f] * r2f[n, r, f]     (elementwise build, DVE)
  P[(d,r,n), t] = sph_d[n, t] * rbf_r[n, t]       (elementwise build, DVE)
Both builds have n on partitions -> matmul contracts partitions directly.
The builds and the matmul run in fp16 (inputs cast host-side), accumulation
stays fp32 in PSUM.

Sharding: split the TARGET axis t (T=2048) across 8 cores (TL=256 each).
The einsum reduces over (n, d, r), not t, so each core's rows are complete -
no all-reduce is needed, only one small AllGather (129KB/rank, row-major
with an explicit zero row appended per rank). The program is compiled per
truncated_idx: each core compacts the ~NR-owned target rows out of the
gathered (T, F) table with a few indirect gathers (duplicate targets pull
their partners from per-level tables, empty slots read the zero row, and
one vector add per level merges them), then scatters the compacted rows
into its NR=8192-row shard of the output with 2-3 indirect DMAs. Pad slots
scatter to a trash row (row NR) that the host drops; scatters alternate
between two output buffers so they pipeline, and the host sums the two.
"""

import sys

sys.path.insert(0, "/opt/trn_rl_repo")

import numpy as np

import concourse.bass as bass
import concourse.mybir as mybir
from concourse import bacc, tile
from concourse.masks import make_identity

A, T, NGRID, RBF, F = 128, 2048, 65536, 16, 128
DS = (1, 3, 5)
NDP = sum(DS)  # 9 d-planes
NCORES = 8
TL = T // NCORES  # 256 targets per core
HL = TL // 2  # 128 targets per half
NR = NGRID // NCORES  # 8192 output rows per core
NTILE = NCORES  # column tiles of 128 per half

F32 = mybir.dt.float32
BF16 = mybir.dt.bfloat16
F16 = mybir.dt.float16
I32 = mybir.dt.int32
MD_MAP = {"f32": F32, "bf16": BF16, "f16": F16}

# dtype of the P/W builds, the big matmul, and MLP layer 3.
MAIN_DTYPE = "f16"
_CACHE: dict = {}


def _build_program(b3_nonzero, main_dtype, n_main, n_lvls, repeats=1):
    md = MD_MAP[main_dtype]
    nc = bacc.Bacc(
        "TRN2", target_bir_lowering=False, debug=False, num_devices=NCORES
    )

    invT_h = nc.dram_tensor("invT", [F, A], F32, kind="ExternalInput")
    wmat_h = nc.dram_tensor("wmat", [6, F, F], F32, kind="ExternalInput")
    bvec_h = nc.dram_tensor("bvec", [6, F, 1], F32, kind="ExternalInput")
    w3_h = nc.dram_tensor("w3", [3, F, RBF * F], md, kind="ExternalInput")
    if b3_nonzero:
        b3_h = nc.dram_tensor("b3", [3, 1, RBF * F], md, kind="ExternalInput")
    featp_h = nc.dram_tensor("featp", [NDP, A, F], md, kind="ExternalInput")
    planes_h = nc.dram_tensor(
        "planes", [NDP + RBF, A, TL], md, kind="ExternalInput"
    )
    gidx_h = nc.dram_tensor(
        "gidx", [128, n_main + n_lvls], I32, kind="ExternalInput"
    )
    soffs_h = nc.dram_tensor("soffs", [128, n_main], I32, kind="ExternalInput")
    out_h = nc.dram_tensor("out", [NR + 1, F], F32, kind="ExternalOutput")
    outb_h = nc.dram_tensor("outb", [NR + 1, F], F32, kind="ExternalOutput")

    with tile.TileContext(nc) as tc:
        with (
            tc.tile_pool(name="const", bufs=1) as const,
            tc.tile_pool(name="mlp", bufs=2) as mlp,
            tc.tile_pool(name="work", bufs=2) as work,
            tc.tile_pool(name="psacc", bufs=1, space="PSUM") as psacc,
            tc.tile_pool(name="pssm", bufs=1, space="PSUM") as pssm,
            tc.tile_pool(name="psbig", bufs=1, space="PSUM") as psbig,
            tc.tile_pool(name="pstr", bufs=2, space="PSUM") as pstr,
            tc.tile_pool(name="dram", bufs=1, space="DRAM") as dram,
        ):
            # ---- input loads, spread across engine DGE queues so the
            # ---- MLP weights, the P-build planes and the rest all load
            # ---- in parallel. planes = (rbf[16], sph[9]) so the P-build
            # ---- can start as soon as rbf + the first sph plane land.
            planes_t = const.tile([A, (NDP + RBF) * TL], md, tag="planes")
            pl3 = planes_t[:].rearrange("p (n t) -> p n t", n=NDP + RBF)
            src3 = planes_h[:].transpose([1, 0, 2])
            nc.sync.dma_start(pl3[:, :RBF, :], src3[:, :RBF, :])
            nc.sync.dma_start(pl3[:, RBF:, :], src3[:, RBF:, :])
            invT_t = const.tile([F, A], F32, tag="invT")
            nc.scalar.dma_start(invT_t[:], invT_h[:])
            wmat_t = const.tile([F, 6 * F], F32, tag="wmat")
            nc.scalar.dma_start(
                wmat_t[:].rearrange("p (i j) -> p i j", i=6),
                wmat_h[:].transpose([1, 0, 2]),
            )
            bvec_t = const.tile([F, 6], F32, tag="bvec")
            nc.scalar.dma_start(
                bvec_t[:].rearrange("p (i j) -> p i j", i=6),
                bvec_h[:].transpose([1, 0, 2]),
            )
            w3_t = const.tile([F, 3 * RBF * F], md, tag="w3")
            nc.scalar.dma_start(
                w3_t[:].rearrange("p (i j) -> p i j", i=3),
                w3_h[:].transpose([1, 0, 2]),
            )
            featp_t = const.tile([A, NDP * F], md, tag="featp")
            nc.gpsimd.dma_start(
                featp_t[:].rearrange("p (n f) -> p n f", n=NDP),
                featp_h[:].transpose([1, 0, 2]),
            )
            gidx_t = const.tile([128, n_main + n_lvls], I32, tag="gidx")
            nc.gpsimd.dma_start(gidx_t[:], gidx_h[:])
            soffs_t = const.tile([128, n_main], I32, tag="soffs")
            nc.gpsimd.dma_start(soffs_t[:], soffs_h[:])
            zrow_t = const.tile([16, F], F32, tag="zrow")
            nc.vector.memset(zrow_t[:], 0.0)
            if b3_nonzero:
                b3_t = const.tile([1, 3 * RBF * F], md, tag="b3")
                nc.scalar.dma_start(
                    b3_t[:].rearrange("p (i j) -> p i j", i=3),
                    b3_h[:].transpose([1, 0, 2]),
                )
                ones_t = const.tile([1, A], md, tag="ones")
                nc.gpsimd.memset(ones_t[:], 1.0)
            ident = const.tile([128, 128], F32, tag="ident")
            make_identity(nc, ident[:])

            def sph(dg):
                return planes_t[:, (RBF + dg) * TL:(RBF + dg + 1) * TL]

            def rbf_all():
                return planes_t[:, :RBF * TL].rearrange(
                    "p (r t) -> p r t", r=RBF
                )

            for _rep in range(repeats):
                # ---- MLP (per irrep): r2f = silu(silu(inv@w1+b1)@w2+b2)@w3
                r2f_t = []
                for i in range(3):
                    h1p = pssm.tile([F, A], F32, tag="hsm", bufs=2, name="h1p")
                    nc.tensor.matmul(
                        h1p[:], wmat_t[:, i * F:(i + 1) * F], invT_t[:],
                        start=True, stop=True,
                    )
                    h1 = mlp.tile([F, A], F32, tag="h1")
                    nc.scalar.activation(
                        h1[:], h1p[:], mybir.ActivationFunctionType.Silu,
                        bias=bvec_t[:, i:i + 1],
                    )
                    h2p = pssm.tile([F, A], F32, tag="hsm", bufs=2, name="h2p")
                    nc.tensor.matmul(
                        h2p[:], wmat_t[:, (3 + i) * F:(4 + i) * F], h1[:],
                        start=True, stop=True,
                    )
                    h2 = mlp.tile([F, A], md, tag="h2")
                    nc.scalar.activation(
                        h2[:], h2p[:], mybir.ActivationFunctionType.Silu,
                        bias=bvec_t[:, 3 + i:4 + i],
                    )
                    r2f = mlp.tile([A, RBF * F], md, tag=f"r2f_{i}")
                    for jh in range(2):
                        r2fp = psbig.tile([A, 1024], F32, tag="r2fp")
                        for j in range(2):
                            sl = slice(
                                i * RBF * F + jh * 1024 + j * 512,
                                i * RBF * F + jh * 1024 + (j + 1) * 512,
                            )
                            psl = slice(j * 512, (j + 1) * 512)
                            nc.tensor.matmul(
                                r2fp[:, psl], h2[:], w3_t[:, sl],
                                start=True, stop=not b3_nonzero,
                            )
                            if b3_nonzero:
                                nc.tensor.matmul(
                                    r2fp[:, psl], ones_t[:],
                                    b3_t[:, i * RBF * F + jh * 1024 + j * 512:
                                         i * RBF * F + jh * 1024 + (j + 1) * 512],
                                    start=False, stop=True,
                                )
                        nc.scalar.activation(
                            r2f[:, jh * 1024:(jh + 1) * 1024], r2fp[:],
                            mybir.ActivationFunctionType.Copy,
                        )
                    r2f_t.append(r2f)

                # ---- DVE: build all P / W blocks (resident) ----
                pb_t, wb_t = [], []
                dg = 0
                for i in range(3):
                    for d in range(DS[i]):
                        pb = const.tile(
                            [A, RBF, TL], md, tag=f"pb{dg}", name=f"pb{dg}"
                        )
                        nc.vector.tensor_mul(
                            pb[:], rbf_all(),
                            sph(dg).unsqueeze(1).broadcast_to([A, RBF, TL]),
                        )
                        wb = const.tile(
                            [A, RBF, F], md, tag=f"wb{dg}", name=f"wb{dg}"
                        )
                        nc.vector.tensor_mul(
                            wb[:],
                            r2f_t[i][:].rearrange("p (r f) -> p r f", r=RBF),
                            featp_t[:, dg * F:(dg + 1) * F]
                            .unsqueeze(1).broadcast_to([A, RBF, F]),
                        )
                        pb_t.append(pb)
                        wb_t.append(wb)
                        dg += 1

                # ---- PE: single-pass contraction (144 matmuls, N=256) ----
                acc = psacc.tile([F, TL], F32, tag="acc")
                it = 0
                for dg in range(NDP):
                    for r in range(RBF):
                        nc.tensor.matmul(
                            acc[:],
                            wb_t[dg][:, r, :],
                            pb_t[dg][:, r, :],
                            start=(it == 0), stop=(it == NDP * RBF - 1),
                        )
                        it += 1
                accs = work.tile([F, TL], F32, tag="accs")
                nc.scalar.activation(
                    accs[:], acc[:], mybir.ActivationFunctionType.Copy
                )

                # ---- local transpose to row-major + zero row, then AG ----
                agin = dram.tile([TL + 16, F], F32, name="agin")
                for hh in range(2):
                    trp = pstr.tile([128, 128], F32, tag="trp", bufs=2)
                    nc.tensor.transpose(
                        trp[:], accs[:, hh * 128:(hh + 1) * 128], ident[:]
                    )
                    rl = work.tile([128, F], F32, tag="rl")
                    nc.scalar.activation(
                        rl[:], trp[:], mybir.ActivationFunctionType.Copy
                    )
                    nc.gpsimd.dma_start(
                        agin[hh * 128:(hh + 1) * 128, :], rl[:]
                    )
                nc.gpsimd.dma_start(agin[TL:TL + 16, :], zrow_t[:])
                agout = dram.tile([NCORES * (TL + 16), F], F32, name="agout")
                nc.gpsimd.collective_compute(
                    "AllGather",
                    mybir.AluOpType.bypass,
                    replica_groups=[list(range(NCORES))],
                    ins=[agin[:].opt()],
                    outs=[agout[:].opt()],
                )

                # ---- compact owned rows via indirect gathers, merge dups,
                # ---- scatter to the output shard (pads go to trash row NR)
                for g in range(n_main):
                    gm = work.tile([128, F], F32, tag=f"gm{g}", name=f"gm{g}")
                    nc.gpsimd.indirect_dma_start(
                        out=gm[:],
                        out_offset=None,
                        in_=agout[:],
                        in_offset=bass.IndirectOffsetOnAxis(
                            ap=gidx_t[:, g:g + 1], axis=0
                        ),
                    )
                    for l in range(n_lvls if g == 0 else 0):
                        gd = work.tile(
                            [128, F], F32, tag=f"gd{l}", name=f"gd{l}"
                        )
                        nc.gpsimd.indirect_dma_start(
                            out=gd[:],
                            out_offset=None,
                            in_=agout[:],
                            in_offset=bass.IndirectOffsetOnAxis(
                                ap=gidx_t[:, n_main + l:n_main + l + 1],
                                axis=0,
                            ),
                        )
                        nc.vector.tensor_add(gm[:], gm[:], gd[:])
                    nc.gpsimd.indirect_dma_start(
                        out=(out_h if g % 2 == 0 else outb_h)[:],
                        out_offset=bass.IndirectOffsetOnAxis(
                            ap=soffs_t[:, g:g + 1], axis=0
                        ),
                        in_=gm[:],
                        in_offset=None,
                    )

    nc.compile()
    return nc


def _prep(inputs, main_dtype):
    """Host-side input prep -> (per-core in_maps, dup_pairs, b3_nonzero)."""
    if main_dtype == "f32":
        md = np.float32
    elif main_dtype == "f16":
        md = np.float16
    else:
        import ml_dtypes

        md = np.dtype(ml_dtypes.bfloat16)

    f0 = np.asarray(inputs["feat0"], np.float32)
    inv_rbf = np.float32(1.0 / np.sqrt(RBF))

    invT = np.ascontiguousarray(f0[:, :, 0].T)
    w1 = np.asarray(inputs["mlp_w1"], np.float32)
    w2 = np.asarray(inputs["mlp_w2"], np.float32)
    wmat = np.ascontiguousarray(np.concatenate([w1, w2], axis=0))
    b1 = np.asarray(inputs["mlp_b1"], np.float32)
    b2 = np.asarray(inputs["mlp_b2"], np.float32)
    bvec = np.concatenate([b1, b2], axis=0).reshape(6, F, 1).copy()
    w3 = (np.asarray(inputs["mlp_w3"], np.float32) * inv_rbf).astype(md)
    b3f = np.asarray(inputs["mlp_b3"], np.float32) * inv_rbf
    b3_nonzero = bool(np.any(b3f))
    b3 = b3f.reshape(3, 1, RBF * F).astype(md)

    featp = np.concatenate(
        [
            np.asarray(inputs[f"feat{i}"], np.float32).transpose(2, 0, 1)
            for i in range(3)
        ],
        axis=0,
    ).astype(md)  # (9, A, F)
    sphp = np.concatenate(
        [
            np.asarray(inputs[f"sph{i}"], np.float32).transpose(2, 0, 1)
            for i in range(3)
        ],
        axis=0,
    )  # (9, A, T)
    rbfp = np.asarray(inputs["radial_basis_vals"], np.float32).transpose(
        2, 0, 1
    )  # (RBF, A, T)
    planes = np.concatenate([rbfp, sphp], axis=0).astype(md)  # (25, A, T)

    idx = np.asarray(inputs["truncated_idx"]).astype(np.int64)
    first: dict = {}
    dups_of: dict = {}
    for t, v in enumerate(idx.tolist()):
        if v in first:
            dups_of.setdefault(v, []).append(t)
        else:
            first[v] = t
    n_lvls = max((len(x) for x in dups_of.values()), default=0)

    BLK = TL + 16  # 272-row AG blocks (16 zero rows pad to CCE slices)

    def agrow(t):
        return (t // TL) * BLK + (t % TL)

    ZROW = TL  # block-0 zero row in agout
    per_core_owned = []
    for c in range(NCORES):
        lo, hi = c * NR, (c + 1) * NR
        per_core_owned.append(sorted(
            (v for v in first if lo <= v < hi),
            key=lambda v: (0 if v in dups_of else 1, v),
        ))
    n_main = max(1, max((len(o) + 127) // 128 for o in per_core_owned))

    in_maps = []
    for c in range(NCORES):
        owned = per_core_owned[c]
        gidx = np.zeros((128, n_main + n_lvls), np.int32)
        gidx[:, n_main:] = ZROW  # dup tables default to the zero row
        soffs = np.full((128, n_main), NR, np.int32)  # pads -> trash row
        for s, v in enumerate(owned):
            g, p = divmod(s, 128)
            gidx[p, g] = agrow(first[v])
            soffs[p, g] = v - c * NR
            for l, td in enumerate(dups_of.get(v, ())):
                assert g == 0, "dup leaders must pack into gather slot 0"
                gidx[p, n_main + l] = agrow(td)
        ts = slice(c * TL, (c + 1) * TL)
        m = {
            "invT": invT,
            "wmat": wmat, "bvec": bvec, "w3": w3,
            "featp": featp,
            "planes": np.ascontiguousarray(planes[:, :, ts]),
            "gidx": gidx,
            "soffs": soffs,
        }
        if b3_nonzero:
            m["b3"] = b3
        in_maps.append(m)
    return in_maps, (n_main, n_lvls), b3_nonzero



def _get_runner(meta, b3_nonzero, main_dtype, repeats=1):
    n_main, n_lvls = meta
    key = (meta, b3_nonzero, main_dtype, repeats)
    if key not in _CACHE:
        nc = _build_program(b3_nonzero, main_dtype, n_main, n_lvls, repeats)
        _CACHE[key] = nc
    return _CACHE[key]


def run_on_hw(in_maps, nc):
    from concourse import bass_utils

    res = bass_utils.run_bass_kernel_spmd(
        nc, in_maps, core_ids=list(range(NCORES))
    )
    return res.results


def kernel(**inputs) -> np.ndarray:
    in_maps, meta, b3_nonzero = _prep(inputs, MAIN_DTYPE)
    nc = _get_runner(meta, b3_nonzero, MAIN_DTYPE)
    results = run_on_hw(in_maps, nc)
    return np.concatenate(
        [
            results[c]["out"][:NR] + results[c]["outb"][:NR]
            for c in range(NCORES)
        ],
        axis=0,
    )
